# revision 3
# baseline (speedup 1.0000x reference)
"""Trainium2 Bass kernel for nn_Attention_45724221833663 (sparse_attention).

Strategy: data-parallel over batch B=8 across the 8 NeuronCores (one batch
element per core). All matmuls run in bf16 with fp32 PSUM accumulation.

Per-core dataflow (all layouts chosen to avoid on-chip transposes of large
activations; weights and x are transposed on the host while sharding):
  xcatT  [c=1024, kvp=1152]  (= concat(x_text, x).T, zero-padded 1101->1152)
  qT     [o, n]    = WqT.T @ xT          (o = head-major channel)
  kT     [o, kvp]  = WkT.T @ xcatT
  vw     [kvp, h, 65] = (xcatT.T @ WvT) interleaved per head + ones column
  per head h:
    scoresT[kv, n] = kT_h.T-contracted with qT_h  (K=d=64 contraction)
    E = exp(scoresT / 8)     (ScalarE, psum -> sbuf bf16); row kv=0 and the
                             pad rows are zeroed
    avp[n,0:65] = sum_kv E[kv,n-tile] * vw[kv, h, :]   (col 64 = S[n])
    attn[n, h*64:+64] = avp[:, :64] * (1/S) + tanh(g_h) * v_h[kv=0]
  LayerNorm over channels (rows of attn, bf16 input like the reference's
  bf16 cast), then out = LN @ Wp.T + bp via PE-transposed LN tiles.
"""

import os
import numpy as np
import ml_dtypes

import concourse.bass as bass
import concourse.bacc as bacc
import concourse.tile as tile
from concourse import mybir
from concourse.masks import make_identity
from concourse.bass_utils import run_bass_kernel_spmd

F32 = mybir.dt.float32
BF16 = mybir.dt.bfloat16
AF = mybir.ActivationFunctionType
OP = mybir.AluOpType

B, N, P, DIM, H = 8, 1024, 77, 1024, 16
HD = DIM // H          # 64
KV = P + N             # 1101
KT = 9                 # kv tiles of 128
KVP = KT * 128         # 1152 padded
NT = N // 128          # 8 n tiles
CC = DIM // 128        # 8 contraction chunks
OT = DIM // 128        # 8 output-channel tiles
LN_EPS = 1e-5

LAST_EXEC_NS = None
_CACHE = {}


def _emit(tc):
    nc = tc.nc

    xcatT_d = nc.dram_tensor("xcatT", [DIM, KVP], BF16, kind="ExternalInput").ap()
    wq_d = nc.dram_tensor("wqT", [DIM, DIM], BF16, kind="ExternalInput").ap()
    wk_d = nc.dram_tensor("wkT", [DIM, DIM], BF16, kind="ExternalInput").ap()
    wv_d = nc.dram_tensor("wvT", [DIM, DIM], BF16, kind="ExternalInput").ap()
    wp_d = nc.dram_tensor("wpT", [DIM, DIM], BF16, kind="ExternalInput").ap()
    tanhg_d = nc.dram_tensor("tanhg", [1, H], F32, kind="ExternalInput").ap()
    lng_d = nc.dram_tensor("ln_g", [1, DIM], F32, kind="ExternalInput").ap()
    lnb_d = nc.dram_tensor("ln_b", [1, DIM], F32, kind="ExternalInput").ap()
    bp_d = nc.dram_tensor("bp", [1, DIM], F32, kind="ExternalInput").ap()
    out_d = nc.dram_tensor("out", [N, DIM], F32, kind="ExternalOutput").ap()

    from contextlib import ExitStack

    with ExitStack() as top:
        consts = top.enter_context(tc.tile_pool(name="consts", bufs=1))
        acts = top.enter_context(tc.tile_pool(name="acts", bufs=1))
        tpool = top.enter_context(tc.tile_pool(name="tmp", bufs=3))
        opool = top.enter_context(tc.tile_pool(name="outp", bufs=2))

        # ---- constants ----
        tanhg_sb = consts.tile([128, H], F32, tag="tanhg")
        nc.sync.dma_start(out=tanhg_sb, in_=tanhg_d.to_broadcast([128, H]))
        g_b = consts.tile([128, DIM], F32, tag="g_b")
        nc.sync.dma_start(out=g_b, in_=lng_d.to_broadcast([128, DIM]))
        b_b = consts.tile([128, DIM], F32, tag="b_b")
        nc.sync.dma_start(out=b_b, in_=lnb_d.to_broadcast([128, DIM]))
        bp_b = consts.tile([128, DIM], F32, tag="bp_b")
        nc.sync.dma_start(out=bp_b, in_=bp_d.to_broadcast([128, DIM]))
        eps_t = consts.tile([128, 1], F32, tag="eps")
        nc.vector.memset(eps_t, LN_EPS)
        ident = consts.tile([128, 128], BF16, tag="ident")
        make_identity(nc, ident)

        # ---- persistent activations ----
        qT_sb = acts.tile([128, OT, N], BF16, tag="qT")        # [o-part, o-tile, n]
        kT_sb = acts.tile([128, OT, KVP], BF16, tag="kT")      # [o-part, o-tile, kv]
        vw_sb = acts.tile([128, KT, H, HD + 1], BF16, tag="vw")  # [kv-part, kv-tile, h, d+1]
        attn_sb = acts.tile([128, NT, H, HD], BF16, tag="attn")  # [n-part, n-tile, h, d]
        L_sb = acts.tile([128, NT, DIM], BF16, tag="L")        # LN output [n-part, n-tile, c]
        wp_sb = acts.tile([128, CC, DIM], BF16, tag="wp")      # [c-part, c-chunk, o]
        nc.sync.dma_start(out=wp_sb, in_=wp_d.rearrange("(j p) o -> p j o", p=128))

        # ================= phase 1: projections =================
        with tc.tile_pool(name="ph1", bufs=1) as ph1, \
             tc.tile_pool(name="wstream", bufs=2) as wstream, \
             tc.tile_pool(name="ps_proj", bufs=4, space="PSUM") as ps_proj:
            xcatT_sb = ph1.tile([128, CC, KVP], BF16, tag="xcatT")
            nc.sync.dma_start(
                out=xcatT_sb, in_=xcatT_d.rearrange("(j p) f -> p j f", p=128)
            )
            wv_sb = ph1.tile([128, CC, DIM], BF16, tag="wv")
            nc.sync.dma_start(out=wv_sb, in_=wv_d.rearrange("(j p) o -> p j o", p=128))

            wq_re = wq_d.rearrange("(j p) o -> p j o", p=128)
            wk_re = wk_d.rearrange("(j p) o -> p j o", p=128)

            # q projection: qT[o, n]
            for ot in range(OT):
                wt = wstream.tile([128, CC, 128], BF16, tag="wq")
                nc.sync.dma_start(out=wt, in_=wq_re[:, :, ot * 128:(ot + 1) * 128])
                for half in range(2):
                    ps = ps_proj.tile([128, 512], F32, tag="ps")
                    for cc in range(CC):
                        nc.tensor.matmul(
                            ps,
                            wt[:, cc, :],
                            xcatT_sb[:, cc, P + half * 512: P + (half + 1) * 512],
                            start=(cc == 0),
                            stop=(cc == CC - 1),
                        )
                    nc.vector.tensor_copy(
                        qT_sb[:, ot, half * 512:(half + 1) * 512], ps
                    )

            # k projection: kT[o, kvp]
            ksplits = [(0, 512), (512, 512), (1024, 128)]
            for ot in range(OT):
                wt = wstream.tile([128, CC, 128], BF16, tag="wk")
                nc.sync.dma_start(out=wt, in_=wk_re[:, :, ot * 128:(ot + 1) * 128])
                for off, width in ksplits:
                    ps = ps_proj.tile([128, 512], F32, tag="ps")
                    for cc in range(CC):
                        nc.tensor.matmul(
                            ps[:, :width],
                            wt[:, cc, :],
                            xcatT_sb[:, cc, off:off + width],
                            start=(cc == 0),
                            stop=(cc == CC - 1),
                        )
                    nc.vector.tensor_copy(kT_sb[:, ot, off:off + width], ps[:, :width])

            # v projection into vw (head-interleaved), natural [kv, o] layout
            for kvt in range(KT):
                for half in range(2):
                    ps = ps_proj.tile([128, 512], F32, tag="ps")
                    for cc in range(CC):
                        nc.tensor.matmul(
                            ps,
                            xcatT_sb[:, cc, kvt * 128:(kvt + 1) * 128],
                            wv_sb[:, cc, half * 512:(half + 1) * 512],
                            start=(cc == 0),
                            stop=(cc == CC - 1),
                        )
                    nc.vector.tensor_copy(
                        vw_sb[:, kvt, half * 8:(half + 1) * 8, 0:HD],
                        ps.rearrange("p (h d) -> p h d", d=HD),
                    )
            # ones column for the row-sum S (E rows for kv=0/pad are zeroed)
            nc.vector.memset(vw_sb[:, :, :, HD:HD + 1], 1.0)

        # ================= phase 2: attention per head =================
        with tc.tile_pool(name="epool", bufs=2) as epool, \
             tc.tile_pool(name="ps_scores", bufs=2, space="PSUM") as ps_scores, \
             tc.tile_pool(name="ps_av", bufs=2, space="PSUM") as ps_av:
            for h in range(H):
                hb = (h % 2) * 64
                hp = h // 2
                e = epool.tile([128, KT, N], BF16, tag="e")
                # pad rows (kv >= 1101) must contribute nothing: zero the last
                # kv tile, then exp only its valid 77 rows
                nc.vector.memset(e[:, KT - 1, :], 0.0)
                last_rows = KV - (KT - 1) * 128  # 77
                for kvt in range(KT):
                    pss = ps_scores.tile([128, N], F32, tag="pss")
                    lhsT = kT_sb[hb:hb + 64, hp, kvt * 128:(kvt + 1) * 128]
                    for half in range(2):
                        nc.tensor.matmul(
                            pss[:, half * 512:(half + 1) * 512],
                            lhsT,
                            qT_sb[hb:hb + 64, hp, half * 512:(half + 1) * 512],
                            start=True,
                            stop=True,
                        )
                    rows = last_rows if kvt == KT - 1 else 128
                    nc.scalar.activation(
                        e[:rows, kvt, :], pss[:rows], AF.Exp, bias=0.0, scale=0.125
                    )
                # first key column is gated separately
                nc.vector.memset(e[0:1, 0, :], 0.0)

                # gate * v[kv=0] broadcast to all partitions
                gv0 = tpool.tile([128, HD], BF16, tag="gv0")
                nc.gpsimd.partition_broadcast(gv0, vw_sb[0:1, 0, h, 0:HD])
                gv0s = tpool.tile([128, HD], F32, tag="gv0s")
                nc.vector.tensor_scalar_mul(gv0s, gv0, tanhg_sb[:, h:h + 1])

                for nt in range(NT):
                    avp = ps_av.tile([128, HD + 1], F32, tag="avp")
                    for kvt in range(KT):
                        nc.tensor.matmul(
                            avp,
                            e[:, kvt, nt * 128:(nt + 1) * 128],
                            vw_sb[:, kvt, h, :],
                            start=(kvt == 0),
                            stop=(kvt == KT - 1),
                        )
                    rs = tpool.tile([128, 1], F32, tag="rs")
                    nc.vector.reciprocal(rs, avp[:, HD:HD + 1])
                    nc.vector.scalar_tensor_tensor(
                        out=attn_sb[:, nt, h, :],
                        in0=avp[:, 0:HD],
                        scalar=rs,
                        in1=gv0s,
                        op0=OP.mult,
                        op1=OP.add,
                    )

        # ================= phase 3: LayerNorm + output projection =================
        with tc.tile_pool(name="ph3", bufs=3) as ph3, \
             tc.tile_pool(name="ps_pp", bufs=2, space="PSUM") as ps_pp, \
             tc.tile_pool(name="ps_t", bufs=2, space="PSUM") as ps_t:
            for nt in range(NT):
                xa = attn_sb[:, nt].rearrange("p h d -> p (h d)")
                xs = xa.rearrange("p (s f) -> p s f", f=512)
                stats = ph3.tile([128, 2, 6], F32, tag="stats")
                for s in range(2):
                    nc.vector.bn_stats(stats[:, s, :], xs[:, s, :])
                mv = ph3.tile([128, 2], F32, tag="mv")
                nc.vector.bn_aggr(mv, stats)
                rstd = ph3.tile([128, 1], F32, tag="rstd")
                nc.scalar.activation(rstd, mv[:, 1:2], AF.Sqrt, bias=eps_t, scale=1.0)
                nc.vector.reciprocal(rstd, rstd)
                t = ph3.tile([128, DIM], F32, tag="t")
                nc.vector.tensor_scalar(
                    out=t, in0=xa, scalar1=mv[:, 0:1], scalar2=rstd,
                    op0=OP.subtract, op1=OP.mult,
                )
                tg = ph3.tile([128, DIM], F32, tag="tg")
                nc.vector.tensor_tensor(out=tg, in0=t, in1=g_b, op=OP.mult)
                nc.vector.tensor_tensor(out=L_sb[:, nt, :], in0=tg, in1=b_b, op=OP.add)

                # transpose LN rows then project: out[n, o] = L @ Wp.T
                pp0 = ps_pp.tile([128, 512], F32, tag="pp")
                pp1 = ps_pp.tile([128, 512], F32, tag="pp")
                for cc in range(CC):
                    pst = ps_t.tile([128, 128], BF16, tag="pst")
                    nc.tensor.transpose(
                        pst, L_sb[:, nt, cc * 128:(cc + 1) * 128], ident
                    )
                    ltc = ph3.tile([128, 128], BF16, tag="ltc")
                    nc.vector.tensor_copy(ltc, pst)
                    nc.tensor.matmul(
                        pp0, ltc, wp_sb[:, cc, 0:512],
                        start=(cc == 0), stop=(cc == CC - 1),
                    )
                    nc.tensor.matmul(
                        pp1, ltc, wp_sb[:, cc, 512:1024],
                        start=(cc == 0), stop=(cc == CC - 1),
                    )
                ot_t = opool.tile([128, DIM], F32, tag="ot")
                nc.vector.tensor_tensor(out=ot_t[:, 0:512], in0=pp0, in1=bp_b[:, 0:512], op=OP.add)
                nc.vector.tensor_tensor(out=ot_t[:, 512:1024], in0=pp1, in1=bp_b[:, 512:1024], op=OP.add)
                nc.sync.dma_start(out=out_d[nt * 128:(nt + 1) * 128, :], in_=ot_t)


def build_program():
    if "nc" in _CACHE:
        return _CACHE["nc"]
    nc = bacc.Bacc("TRN2", target_bir_lowering=False, debug=False, num_devices=8)
    with tile.TileContext(nc) as tc:
        _emit(tc)
    nc.compile()
    _CACHE["nc"] = nc
    return nc


def prep_inputs(x, x_text, Wq, Wk, Wv, gate, ln_g, ln_b, Wp, bp):
    """Host-side sharding/layout prep. Returns the 8 per-core input maps."""
    bf = ml_dtypes.bfloat16
    x = np.asarray(x, np.float32)
    x_text = np.asarray(x_text, np.float32)
    xcat = np.concatenate([x_text, x], axis=1)          # [B, KV, DIM]
    xcatT = np.zeros((B, DIM, KVP), np.float32)
    xcatT[:, :, :KV] = xcat.transpose(0, 2, 1)
    xcatT = xcatT.astype(bf)
    wqT = np.ascontiguousarray(np.asarray(Wq, np.float32).T).astype(bf)
    wkT = np.ascontiguousarray(np.asarray(Wk, np.float32).T).astype(bf)
    wvT = np.ascontiguousarray(np.asarray(Wv, np.float32).T).astype(bf)
    wpT = np.ascontiguousarray(np.asarray(Wp, np.float32).T).astype(bf)
    tanhg = np.tanh(np.asarray(gate, np.float32)).reshape(1, H).astype(np.float32)
    lng = np.asarray(ln_g, np.float32).reshape(1, DIM)
    lnb = np.asarray(ln_b, np.float32).reshape(1, DIM)
    bpv = np.asarray(bp, np.float32).reshape(1, DIM)
    in_maps = []
    for b in range(B):
        in_maps.append({
            "xcatT": np.ascontiguousarray(xcatT[b]),
            "wqT": wqT, "wkT": wkT, "wvT": wvT, "wpT": wpT,
            "tanhg": tanhg, "ln_g": lng, "ln_b": lnb, "bp": bpv,
        })
    return in_maps


def kernel(**inputs):
    global LAST_EXEC_NS
    nc = build_program()
    in_maps = prep_inputs(**inputs)
    trace = bool(int(os.environ.get("BASS_TRACE_RUN", "0")))
    res = run_bass_kernel_spmd(
        nc, in_maps, core_ids=list(range(8)), trace=trace,
    )
    LAST_EXEC_NS = res.exec_time_ns
    out = np.stack([r["out"] for r in res.results], axis=0)
    return out.astype(np.float32)
